# revision 45
# baseline (speedup 1.0000x reference)
"""Expert-parallel MoE routing kernel for Trainium2 (8 NeuronCores).

Problem: top-k(=2) softmax-gated MoE FFN (relu), followed by
log_softmax(sum(moe_out, axis=-1)) over the sequence dim.

Key algebraic observation: the graded output is
    log_softmax_S( sum_d moe_out[t, d] )
and
    sum_d moe_out[t, :] = sum_e g[t,e] * (relu(x_t @ W1_e + b1_e) @ rowsum(W2_e) + sum(b2_e))
so the entire second expert matmul collapses to a matvec against
s_e = rowsum(W2_e), which the host computes once (a single pass over w2);
w2 (8 MB/core in the baseline) never touches the device.  The gate values
are folded into the dispatched tokens on the host (g * relu(x@W1) ==
relu((g*x)@W1) for g > 0 when b1 == 0; with b1 != 0 the gate is applied
on the host instead).

Per-core device program (core e owns expert e):
  PE : h_pre = xtg^T @ W1 accumulated over D.  Chunk 0 is all-bf16
       (phase A is DMA-paced and has no headroom for extra bytes).  For
       chunks >= 1, the first KQ(=2) k-tiles run as ONE DoubleRow fp8
       matmul per m-tile (two 128-deep planes per instruction at 1
       column-pair/cycle) — each replaces two bf16 matmuls, cutting
       NG*grp PE slots per later chunk (~1.9us for NC=2).  Scale
       matching inside a chunk's PSUM: fp8 planes carry SX (tokens) and
       SW (weights); the chunk's bf16 tokens are pre-scaled by SX*SW
       (exact, power of 2) against unscaled shared w1, so every matmul
       contributes SX*SW*(x.w); the host folds 1/(SX*SW) into that
       chunk's w2 row-sum columns.
  ACT: relu(h_pre + b1) per [128, ln] m-tile, PSUM -> SBUF
  DVE: acc += relu_h * s_m  (signed per-partition scale, fused mult-add)
  PE : z = ones^T @ acc     (final 128-partition reduction)
Host gathers z per core, scatter-adds into [T], applies log_softmax.

fp8 numerics: TRN fp8_e4 == OCP e4m3fn bit-for-bit in (-240, 240); e4m3
noise on KQ/KD of the contraction for later-chunk tokens costs ~9e-3
rel err (budget 2e-2; all-bf16 baseline 1.7e-3; kq=4 measured 2.04e-2).

Timing notes (measured): ~6.6 us fixed prologue + ~9.4 us fixed ucode
teardown bracket the kernel; a scratch-data PE warmup during the
prologue flips the HAM clock-gate to 2.4 GHz before the real stream and
covers the worst-case ~14.5 us first-piece DMA post time (per-piece
completion receipt is ~2.2-4.6 us and jittery across cores — every
scheme that started the real stream earlier lost on max-over-cores when
one core's gap re-throttled the HAM; NWARM=32 measured tighter worst-
core than 28).  Dead ends that measured WORSE: bulk loads on the ACT
HWDGE ring (posts 6-9 us late), gpsimd SWDGE bulk loads (+14 us),
kd-split first pieces, fp8-only head-start groups (PE idles at the bf16
catch-up point).  Sustained back-to-back benching pushes the chip into
the P0 power state (PE ~2.0 GHz, warm matmul 141 ns vs 117) — compare
runs by MM spacing, not absolute totals.
"""

import os

import numpy as np

N_CORES = 8
P = 128
GRP = 2  # m-tiles per w1 column-group (one 256-col block per (group, kd))
NWARM = int(os.environ.get("MOE_NWARM", "31"))
KQ = int(os.environ.get("MOE_KQ", "2"))  # fp8 DoubleRow k-tiles on chunks >= 1
SX = 16.0    # power-of-2 scale on fp8 tokens  (xg max ~5 -> ~80, fp8e4 max 240)
SW = 512.0   # power-of-2 scale on fp8 weights (w1 max ~0.18 -> ~92)


def _round_up(v, m):
    return ((v + m - 1) // m) * m


_BUILD_CACHE = {}


def _grp_for(NC):
    # chunk-phase-major loop: grp accumulators + 1 warmup tile per group in
    # flight, independent of NC — GRP=2 always fits the 6-buf PSUM pool
    return GRP


def _build_program(D, H, ln, NC, kq):
    """Trace + compile the single-core program (SPMD across 8 cores).

    Per-core inputs (KB = KD - kq; chunk 0 keeps all KD k-tiles in bf16,
    chunks c >= 1 keep k-tiles kq..KD-1 in bf16 pre-scaled by SX*SW):
      xtg [P, (KD+(NC-1)*KB)*ln] bf16  chunk-0 block kd at kd*ln; chunk-c
                                 (c>=1) block kd' at (KD+(c-1)*KB+kd'-kq)*ln
      w1  [P, KD*H]      bf16   shared weights; block (g, kd) of GP cols at
                                (g*KD+kd)*GP
      xq  [P, NC-1, kq, ln] fp8 later chunks' tokens * SX for k-tiles < kq
      wq  [P, MH, kq, P]    fp8 weights * SW for k-tiles < kq, plane (m, kd)
      sm  [P, 3*MH+1+grp] f32  [0:MH) w2 row-sums (col m = s[m*P:(m+1)*P]),
                               [MH:2MH) b1 tiled likewise, [2MH] ones,
                               [2MH+1:3MH+1) row-sums / (SX*SW) for chunks>=1,
                               [3MH+1:] unused pad (alignment convenience)
    Output:
      z [1, NC*ln] f32  z[c] = sum_h s_h * relu(x_c @ w1_h + b1_h)
    """
    key = (D, H, ln, NC, kq)
    if key in _BUILD_CACHE:
        return _BUILD_CACHE[key]

    import concourse.tile as tile
    from concourse import bacc, mybir

    f32 = mybir.dt.float32
    f32r = mybir.dt.float32r
    bf16 = mybir.dt.bfloat16
    fp8 = mybir.dt.float8e4
    KD = D // P   # k-tiles over D
    MH = H // P   # m-tiles over H
    KB = KD - kq  # bf16 k-tiles on later chunks
    grp = _grp_for(NC)
    NG = MH // grp  # w1 column groups
    GP = grp * P  # columns per w1 block
    assert kq % 2 == 0 and 0 <= kq < KD
    use_q = kq > 0
    XW = KD + (NC - 1) * (KB if use_q else KD)  # xtg k-tile blocks total

    nc = bacc.Bacc("TRN2", target_bir_lowering=False, debug=False)
    xtg_d = nc.dram_tensor("xtg", [P, XW * ln], bf16, kind="ExternalInput").ap()
    w1_d = nc.dram_tensor("w1", [P, KD * H], bf16, kind="ExternalInput").ap()
    sm_d = nc.dram_tensor("sm", [P, 3 * MH + 1 + grp], f32, kind="ExternalInput").ap()
    if use_q:
        xq_d = nc.dram_tensor("xq", [P, NC, kq, ln], fp8, kind="ExternalInput").ap()
        wq_d = nc.dram_tensor("wq", [P, MH, kq, P], fp8, kind="ExternalInput").ap()
    z_d = nc.dram_tensor("z", [1, NC * ln], f32, kind="ExternalOutput").ap()

    def xtg_block(c, kd):
        # column offset of (chunk c, k-tile kd) in xtg
        if c == 0 or not use_q:
            return (c * KD + kd) * ln
        return (KD + (c - 1) * KB + (kd - kq)) * ln

    with tile.TileContext(nc) as tc:
        with (
            tc.tile_pool(name="persist", bufs=1) as persist,
            tc.tile_pool(name="ht", bufs=6) as htp,
            tc.tile_pool(name="psum_h", bufs=6, space="PSUM") as psum_h,
            tc.tile_pool(name="psum_z", bufs=2, space="PSUM") as psum_z,
        ):
            # --- small loads first: w2 row-sums / b1 / ones ---
            # (smalls ride the ACT HWDGE ring so they never queue behind the
            # bulk xtg/w1 stream on the SP ring)
            sm_sb = persist.tile([P, 3 * MH + 1 + grp], f32)
            nc.scalar.dma_start(out=sm_sb[:], in_=sm_d[:])
            w2s = sm_sb[:, 0:MH]
            b1t = sm_sb[:, MH : 2 * MH]
            w2s_c1 = sm_sb[:, 2 * MH + 1 : 3 * MH + 1]  # descale for chunks >= 1
            ones = persist.tile([P, 1], f32r)
            nc.vector.tensor_copy(out=ones[:], in_=sm_sb[:, 2 * MH : 2 * MH + 1])
            # f32r copies of the last group's w2 row-sums (PE matvec operand):
            # col pair [0:grp) for chunk 0 (raw), [grp:2grp) for chunks >= 1
            # (descaled) — plain f32 engages the 4x-slow fp32 PE mode
            w2sr = persist.tile([P, 2 * grp], f32r)
            nc.vector.tensor_copy(out=w2sr[:, 0:grp], in_=sm_sb[:, MH - grp : MH])
            nc.vector.tensor_copy(
                out=w2sr[:, grp : 2 * grp], in_=sm_sb[:, 3 * MH + 1 - grp : 3 * MH + 1]
            )

            # acc tiles — f32r so the final PE matvec accepts them; two
            # independent accumulation chains (even/odd m) per chunk halve
            # the serial DVE tail.  g == 0 writes them fresh (no memset:
            # walrus rejects f32r memset).
            acc = [
                [
                    persist.tile([P, ln], f32r, tag=f"acc{c}_{p}", name=f"acc{c}_{p}")
                    for p in range(grp)
                ]
                for c in range(NC)
            ] if NG > 1 else None

            # --- weights + activations ---
            xtg_sb = persist.tile([P, XW * ln], bf16)
            w1_sb = persist.tile([P, KD * H], bf16)
            if use_q:
                xq_sb = persist.tile([P, NC, kq, ln], fp8)
                wq_sb = persist.tile([P, MH, kq, P], fp8)

            def w1_load(g):
                sl = slice(g * KD * GP, (g + 1) * KD * GP)
                nc.sync.dma_start(out=w1_sb[:, sl], in_=w1_d[:, sl])

            # PE warmup: dependency-free matmuls on scratch data run during
            # the fixed ~6.6us program prologue, flipping the HAM clock-gate
            # to 2.4GHz and covering the DMA wait for the first real pieces
            if NWARM:
                warm = persist.tile([P, ln], bf16, tag="warm", name="warm")
                nc.vector.memset(warm[:], 0.0)
                pwarm = psum_h.tile([P, ln], f32, tag="psh", name="psh")
                for i in range(NWARM):
                    nc.tensor.matmul(
                        pwarm[:],
                        warm[:, 0:P],
                        warm[:],
                        start=(i == 0),
                        stop=(i == NWARM - 1),
                        skip_group_check=True,
                    )

            # each dma_start costs ~650ns of serial HWDGE issue time and a
            # ~2.2-4.6us completion receipt, and queued transfers drain
            # FIFO — few big pieces, most-critical first, all on the SP
            # ring.  Phase A (chunk 0) is gated by xtg chunk 0 + w1 group-
            # by-group; the fp8 planes + later chunks' pieces ride behind
            # and post during phase A (needed only from ~28us).
            # the two phase-A front pieces go down in halves: on cores whose
            # DMA path has a slow episode (~1.5x wire+receipt, seen on 1-2
            # cores in ~half the runs) the first-needed half (k-tiles 0..3)
            # posts ~3us earlier, shrinking the warmup->data gap below the
            # HAM re-throttle threshold
            hx = KD // 2 * ln
            nc.sync.dma_start(out=xtg_sb[:, 0:hx], in_=xtg_d[:, 0:hx])
            hw = KD // 2 * GP
            nc.sync.dma_start(out=w1_sb[:, 0:hw], in_=w1_d[:, 0:hw])
            nc.sync.dma_start(out=xtg_sb[:, hx : KD * ln], in_=xtg_d[:, hx : KD * ln])
            nc.sync.dma_start(out=w1_sb[:, hw : KD * GP], in_=w1_d[:, hw : KD * GP])
            gq = NG // 2  # chunk-0 groups >= gq run DoubleRow
            for g in range(1, min(4, NG)):
                w1_load(g)
            if use_q:
                # fp8 tokens + the g4-5 planes slip into measured phase-A
                # slack just ahead of chunk-0's first DoubleRow group (~22us)
                nc.sync.dma_start(out=xq_sb[:], in_=xq_d[:])
                q4 = (MH // 4) * 2
                nc.sync.dma_start(out=wq_sb[:, q4 : q4 + MH // 4], in_=wq_d[:, q4 : q4 + MH // 4])
            if NG > 4:
                w1_load(4)
            if use_q:
                nc.sync.dma_start(out=wq_sb[:, 3 * MH // 4 :], in_=wq_d[:, 3 * MH // 4 :])
            for g in range(5, NG):
                w1_load(g)
            if use_q:
                nc.sync.dma_start(out=wq_sb[:, 0 : MH // 2], in_=wq_d[:, 0 : MH // 2])
            if NC > 1:
                sl = slice(KD * ln, XW * ln)
                nc.sync.dma_start(out=xtg_sb[:, sl], in_=xtg_d[:, sl])

            # --- mm1 + relu + scaled accumulate, chunk-phase major ---
            z_sb = persist.tile([1, NC * ln], f32)
            for c in range(NC):
                s_all = w2s_c1 if use_q else w2s
                s_red = w2sr[:, grp : 2 * grp] if use_q else w2sr[:, 0:grp]
                pz = psum_z.tile([1, ln], f32, tag="psz", name="psz")
                for g in range(NG):
                    last_g = g == NG - 1
                    q_c = use_q and (c >= 1 or g >= gq)
                    pss = [
                        psum_h.tile([P, ln], f32, tag="psh", name="psh")
                        for _ in range(grp)
                    ]
                    if q_c:
                        for kp in range(kq // 2):
                            for mi in range(grp):
                                nc.tensor.matmul(
                                    pss[mi][:],
                                    wq_sb[:, g * grp + mi, 2 * kp : 2 * kp + 2, :],
                                    xq_sb[:, c, 2 * kp : 2 * kp + 2, :],
                                    start=(kp == 0),
                                    stop=False,
                                    perf_mode=mybir.MatmulPerfMode.DoubleRow,
                                    skip_group_check=True,
                                )
                    kd0 = kq if q_c else 0
                    for kd in range(kd0, KD):
                        base = (g * KD + kd) * GP
                        for mi in range(grp):
                            nc.tensor.matmul(
                                pss[mi][:],
                                w1_sb[:, base + mi * P : base + (mi + 1) * P],
                                xtg_sb[:, xtg_block(c, kd) : xtg_block(c, kd) + ln],
                                start=(kd == kd0 and not q_c),
                                stop=(kd == KD - 1),
                                skip_group_check=True,
                            )
                    if last_g and NG > 1:
                        # the m0..m(NG*grp-3) chains closed during this
                        # group's matmuls — reduce them into pz now, so only
                        # this group's relu outputs remain on the tail
                        for p in range(grp):
                            nc.tensor.matmul(
                                pz[:],
                                ones[:],
                                acc[c][p][:],
                                start=(p == 0),
                                stop=False,
                                skip_group_check=True,
                            )
                    for mi in range(grp):
                        m = g * grp + mi
                        ht = htp.tile([P, ln], f32r, tag="ht", name="ht")
                        if last_g and mi > 0:
                            # the tail's relus serialize on ACT; run this one
                            # on the (idle) DVE in parallel: max(pre + b1, 0)
                            nc.vector.tensor_scalar(
                                out=ht[:],
                                in0=pss[mi][:],
                                scalar1=b1t[:, m : m + 1],
                                scalar2=0.0,
                                op0=mybir.AluOpType.add,
                                op1=mybir.AluOpType.max,
                            )
                        else:
                            nc.scalar.activation(
                                ht[:],
                                pss[mi][:],
                                mybir.ActivationFunctionType.Relu,
                                bias=b1t[:, m : m + 1],
                            )
                        if last_g:
                            # bypass the DVE chain: scale+reduce this m-tile
                            # directly on the PE (w2s column as stationary)
                            nc.tensor.matmul(
                                pz[:],
                                s_red[:, mi : mi + 1],
                                ht[:],
                                start=(NG == 1 and mi == 0),
                                stop=(mi == grp - 1),
                                skip_group_check=True,
                            )
                        elif g == 0:
                            nc.vector.tensor_scalar(
                                out=acc[c][mi][:],
                                in0=ht[:],
                                scalar1=s_all[:, m : m + 1],
                                scalar2=None,
                                op0=mybir.AluOpType.mult,
                            )
                        else:
                            nc.vector.scalar_tensor_tensor(
                                out=acc[c][mi][:],
                                in0=ht[:],
                                scalar=s_all[:, m : m + 1],
                                in1=acc[c][mi][:],
                                op0=mybir.AluOpType.mult,
                                op1=mybir.AluOpType.add,
                            )

                # phase-end store for this chunk (for all but the last chunk
                # this hides under the next phase)
                sl = slice(c * ln, (c + 1) * ln)
                # DVE copy: ACT just ran the last relu, DVE is idle, and the
                # plain copy shaves ~0.2us off the exposed last-chunk tail
                nc.vector.tensor_copy(out=z_sb[:, sl], in_=pz[:])
                nc.sync.dma_start(out=z_d[:, sl], in_=z_sb[:, sl])

    nc.compile()
    _BUILD_CACHE[key] = nc
    return nc


def kernel(x, wg, w1, b1, w2, b2, k):
    import ml_dtypes
    from concourse.bass_utils import run_bass_kernel_spmd

    bf16 = ml_dtypes.bfloat16
    fp8np = ml_dtypes.float8_e4m3  # TRN fp8_e4-compatible bits for |v| <= 240
    x = np.asarray(x)
    wg = np.asarray(wg)
    w1 = np.asarray(w1)
    b1 = np.asarray(b1)
    w2 = np.asarray(w2)
    b2 = np.asarray(b2)
    k = int(k)

    B, S, D = x.shape
    E = wg.shape[1]
    H = w1.shape[2]
    T = B * S
    KD = D // P
    MH = H // P
    assert E == N_CORES, f"expert-parallel layout assumes E == 8, got {E}"

    xf = np.ascontiguousarray(x.reshape(T, D), dtype=np.float32)

    # --- gate + top-k routing (host; needed to build the dispatch shards) ---
    logits = xf @ wg.astype(np.float32)
    logits -= logits.max(axis=1, keepdims=True)
    np.exp(logits, out=logits)
    scores = logits / logits.sum(axis=1, keepdims=True)
    if k >= E:
        topi = np.broadcast_to(np.arange(E, dtype=np.int64), (T, E))
    else:
        topi = np.argpartition(-scores, k, axis=1)[:, :k]
    rows = np.arange(T)[:, None]
    topv = scores[rows, topi]

    # per-expert token lists
    idx_e = []
    val_e = []
    for e in range(E):
        tmask, kpos = np.nonzero(topi == e)
        idx_e.append(tmask)
        val_e.append(topv[tmask, kpos].astype(np.float32))
    max_cnt = max(len(i) for i in idx_e)

    # chunk geometry: NC chunks of ln <= 512 tokens (PSUM bank limit)
    NC = max(1, -(-max_cnt // 512))
    ln = _round_up(-(-max_cnt // NC), 4)
    C = NC * ln
    grp = _grp_for(NC)
    NG = MH // grp
    assert D % P == 0 and H % P == 0 and MH % grp == 0, (D, H)

    fold_gate = not b1.any()  # g*relu(u) == relu(g*u) only when b1 == 0
    kq = KQ if (KQ and fold_gate) else 0  # fp8 scale folding needs b1 == 0
    KB = KD - kq
    use_q = kq > 0
    s_e = w2.astype(np.float32).sum(axis=2)          # [E, H] row-sums
    b2s_e = b2.astype(np.float32).sum(axis=1)        # [E]
    sq = SX * SW if kq else 1.0

    nc = _build_program(D, H, ln, NC, kq)

    in_maps = []
    for e in range(E):
        n_e = len(idx_e[e])
        xg = np.zeros((D, C), dtype=np.float32)
        if n_e:
            cols = xf[idx_e[e]].T
            if fold_gate:
                cols = cols * val_e[e][None, :]
            xg[:, :n_e] = cols
        xg4 = xg.reshape(KD, P, NC, ln).transpose(1, 2, 0, 3)  # [P, NC, KD, ln]
        w14 = (
            w1[e].astype(np.float32).reshape(KD, P, MH, P).transpose(1, 2, 0, 3)
        )  # [P, MH, KD, P]
        # xtg: raw tokens; chunk 0 all KD k-tiles, chunks >= 1 k-tiles kq..
        # (the sq scale rides on the bf16 WEIGHTS, which are group-local,
        # so mixed DR/non-DR groups stay PSUM-scale-consistent)
        if use_q:
            parts = [xg4[:, 0].reshape(P, KD * ln)]
            for c in range(1, NC):
                parts.append(xg4[:, c, kq:].reshape(P, KB * ln))
            xtg = np.ascontiguousarray(np.concatenate(parts, axis=1)).astype(bf16)
        else:
            xtg = np.ascontiguousarray(xg4.reshape(P, NC * KD * ln)).astype(bf16)
        # w1 -> [P, KD*H], block (g, kd) of grp*P columns, scaled by sq
        w1r = np.ascontiguousarray(
            (w14 * sq)
            .reshape(P, NG, grp, KD, P)
            .transpose(0, 1, 3, 2, 4)
            .reshape(P, KD * H)
        ).astype(bf16)
        sm = np.zeros((P, 3 * MH + 1 + grp), dtype=np.float32)
        sm[:, 0:MH] = s_e[e].reshape(MH, P).T / sq
        sm[:, MH : 2 * MH] = b1[e].astype(np.float32).reshape(MH, P).T * sq
        sm[:, 2 * MH] = 1.0
        sm[:, 2 * MH + 1 : 3 * MH + 1] = sm[:, 0:MH]
        m = {"xtg": xtg, "w1": w1r, "sm": sm}
        if use_q:
            xqv = np.clip(xg4[:, :, :kq] * SX, -240.0, 240.0)   # [P, NC, kq, ln]
            wqv = np.clip(w14[:, :, :kq] * SW, -240.0, 240.0)   # [P, MH, kq, P]
            m["xq"] = np.ascontiguousarray(xqv).astype(fp8np)
            m["wq"] = np.ascontiguousarray(wqv).astype(fp8np)
        in_maps.append(m)

    res = run_bass_kernel_spmd(nc, in_maps, core_ids=list(range(N_CORES)))

    # --- combine: scatter-add per-(token, expert) scalars, then log_softmax ---
    s = np.zeros(T, dtype=np.float32)
    for e in range(E):
        n_e = len(idx_e[e])
        if n_e:
            z = res.results[e]["z"][0, :n_e].astype(np.float32)
            if fold_gate:
                s[idx_e[e]] += z
            else:
                s[idx_e[e]] += val_e[e] * z
    if b2s_e.any():
        for e in range(E):
            if len(idx_e[e]):
                s[idx_e[e]] += val_e[e] * b2s_e[e]

    sm = s.reshape(B, S)
    sm = sm - sm.max(axis=1, keepdims=True)
    out = sm - np.log(np.exp(sm).sum(axis=1, keepdims=True))
    return out.astype(np.float32)


# revision 47
# speedup vs baseline: 1.0590x; 1.0590x over previous
"""Expert-parallel MoE routing kernel for Trainium2 (8 NeuronCores).

Problem: top-k(=2) softmax-gated MoE FFN (relu), followed by
log_softmax(sum(moe_out, axis=-1)) over the sequence dim.

Key algebraic observation: the graded output is
    log_softmax_S( sum_d moe_out[t, d] )
and
    sum_d moe_out[t, :] = sum_e g[t,e] * (relu(x_t @ W1_e + b1_e) @ rowsum(W2_e) + sum(b2_e))
so the entire second expert matmul collapses to a matvec against
s_e = rowsum(W2_e), which the host computes once (a single pass over w2);
w2 (8 MB/core in the baseline) never touches the device.  The gate values
are folded into the dispatched tokens on the host (g * relu(x@W1) ==
relu((g*x)@W1) for g > 0 when b1 == 0; with b1 != 0 the gate is applied
on the host instead).

Per-core device program (core e owns expert e):
  PE : h_pre = xtg^T @ W1 accumulated over D.  Chunk 0 is all-bf16
       (phase A is DMA-paced and has no headroom for extra bytes).  For
       chunks >= 1, the first KQ(=2) k-tiles run as ONE DoubleRow fp8
       matmul per m-tile (two 128-deep planes per instruction at 1
       column-pair/cycle) — each replaces two bf16 matmuls, cutting
       NG*grp PE slots per later chunk (~1.9us for NC=2).  Scale
       matching inside a chunk's PSUM: fp8 planes carry SX (tokens) and
       SW (weights); the chunk's bf16 tokens are pre-scaled by SX*SW
       (exact, power of 2) against unscaled shared w1, so every matmul
       contributes SX*SW*(x.w); the host folds 1/(SX*SW) into that
       chunk's w2 row-sum columns.
  ACT: relu(h_pre + b1) per [128, ln] m-tile, PSUM -> SBUF
  DVE: acc += relu_h * s_m  (signed per-partition scale, fused mult-add)
  PE : z = ones^T @ acc     (final 128-partition reduction)
Host gathers z per core, scatter-adds into [T], applies log_softmax.

fp8 numerics: TRN fp8_e4 == OCP e4m3fn bit-for-bit in (-240, 240); e4m3
noise on KQ/KD of the contraction for later-chunk tokens costs ~9e-3
rel err (budget 2e-2; all-bf16 baseline 1.7e-3; kq=4 measured 2.04e-2).

Timing notes (measured): ~6.6 us fixed prologue + ~9.4 us fixed ucode
teardown bracket the kernel; a scratch-data PE warmup during the
prologue flips the HAM clock-gate to 2.4 GHz before the real stream and
covers the worst-case ~14.5 us first-piece DMA post time (per-piece
completion receipt is ~2.2-4.6 us and jittery across cores — every
scheme that started the real stream earlier lost on max-over-cores when
one core's gap re-throttled the HAM; NWARM=32 measured tighter worst-
core than 28).  Dead ends that measured WORSE: bulk loads on the ACT
HWDGE ring (posts 6-9 us late), gpsimd SWDGE bulk loads (+14 us),
kd-split first pieces, fp8-only head-start groups (PE idles at the bf16
catch-up point).  Sustained back-to-back benching pushes the chip into
the P0 power state (PE ~2.0 GHz, warm matmul 141 ns vs 117) — compare
runs by MM spacing, not absolute totals.
"""

import os

import numpy as np

N_CORES = 8
P = 128
GRP = 2  # m-tiles per w1 column-group (one 256-col block per (group, kd))
NWARM = int(os.environ.get("MOE_NWARM", "32"))
KQ = int(os.environ.get("MOE_KQ", "2"))  # fp8 DoubleRow k-tiles on chunks >= 1
SX = 16.0    # power-of-2 scale on fp8 tokens  (xg max ~5 -> ~80, fp8e4 max 240)
SW = 512.0   # power-of-2 scale on fp8 weights (w1 max ~0.18 -> ~92)


def _round_up(v, m):
    return ((v + m - 1) // m) * m


_BUILD_CACHE = {}


def _grp_for(NC):
    # chunk-phase-major loop: grp accumulators + 1 warmup tile per group in
    # flight, independent of NC — GRP=2 always fits the 6-buf PSUM pool
    return GRP


def _build_program(D, H, ln, NC, kq):
    """Trace + compile the single-core program (SPMD across 8 cores).

    Per-core inputs (KB = KD - kq; chunk 0 keeps all KD k-tiles in bf16,
    chunks c >= 1 keep k-tiles kq..KD-1 in bf16 pre-scaled by SX*SW):
      xtg [P, (KD+(NC-1)*KB)*ln] bf16  chunk-0 block kd at kd*ln; chunk-c
                                 (c>=1) block kd' at (KD+(c-1)*KB+kd'-kq)*ln
      w1  [P, KD*H]      bf16   shared weights; block (g, kd) of GP cols at
                                (g*KD+kd)*GP
      xq  [P, NC-1, kq, ln] fp8 later chunks' tokens * SX for k-tiles < kq
      wq  [P, MH, kq, P]    fp8 weights * SW for k-tiles < kq, plane (m, kd)
      sm  [P, 3*MH+1+grp] f32  [0:MH) w2 row-sums (col m = s[m*P:(m+1)*P]),
                               [MH:2MH) b1 tiled likewise, [2MH] ones,
                               [2MH+1:3MH+1) row-sums / (SX*SW) for chunks>=1,
                               [3MH+1:] unused pad (alignment convenience)
    Output:
      z [1, NC*ln] f32  z[c] = sum_h s_h * relu(x_c @ w1_h + b1_h)
    """
    key = (D, H, ln, NC, kq)
    if key in _BUILD_CACHE:
        return _BUILD_CACHE[key]

    import concourse.tile as tile
    from concourse import bacc, mybir

    f32 = mybir.dt.float32
    f32r = mybir.dt.float32r
    bf16 = mybir.dt.bfloat16
    fp8 = mybir.dt.float8e4
    KD = D // P   # k-tiles over D
    MH = H // P   # m-tiles over H
    KB = KD - kq  # bf16 k-tiles on later chunks
    grp = _grp_for(NC)
    NG = MH // grp  # w1 column groups
    GP = grp * P  # columns per w1 block
    assert kq % 2 == 0 and 0 <= kq < KD
    use_q = kq > 0
    XW = KD + (NC - 1) * (KB if use_q else KD)  # xtg k-tile blocks total

    nc = bacc.Bacc("TRN2", target_bir_lowering=False, debug=False)
    xtg_d = nc.dram_tensor("xtg", [P, XW * ln], bf16, kind="ExternalInput").ap()
    w1_d = nc.dram_tensor("w1", [P, KD * H], bf16, kind="ExternalInput").ap()
    sm_d = nc.dram_tensor("sm", [P, 3 * MH + 1 + grp], f32, kind="ExternalInput").ap()
    if use_q:
        xq_d = nc.dram_tensor("xq", [P, NC, kq, ln], fp8, kind="ExternalInput").ap()
        wq_d = nc.dram_tensor("wq", [P, MH, kq, P], fp8, kind="ExternalInput").ap()
    z_d = nc.dram_tensor("z", [1, NC * ln], f32, kind="ExternalOutput").ap()

    def xtg_block(c, kd):
        # column offset of (chunk c, k-tile kd) in xtg
        if c == 0 or not use_q:
            return (c * KD + kd) * ln
        return (KD + (c - 1) * KB + (kd - kq)) * ln

    with tile.TileContext(nc) as tc:
        with (
            tc.tile_pool(name="persist", bufs=1) as persist,
            tc.tile_pool(name="ht", bufs=6) as htp,
            tc.tile_pool(name="psum_h", bufs=6, space="PSUM") as psum_h,
            tc.tile_pool(name="psum_z", bufs=2, space="PSUM") as psum_z,
        ):
            # --- small loads first: w2 row-sums / b1 / ones ---
            # (smalls ride the ACT HWDGE ring so they never queue behind the
            # bulk xtg/w1 stream on the SP ring)
            sm_sb = persist.tile([P, 3 * MH + 1 + grp], f32)
            nc.scalar.dma_start(out=sm_sb[:], in_=sm_d[:])
            w2s = sm_sb[:, 0:MH]
            b1t = sm_sb[:, MH : 2 * MH]
            w2s_c1 = sm_sb[:, 2 * MH + 1 : 3 * MH + 1]  # descale for chunks >= 1
            ones = persist.tile([P, 1], f32r)
            nc.vector.tensor_copy(out=ones[:], in_=sm_sb[:, 2 * MH : 2 * MH + 1])
            # f32r copies of the last group's w2 row-sums (PE matvec operand):
            # col pair [0:grp) for chunk 0 (raw), [grp:2grp) for chunks >= 1
            # (descaled) — plain f32 engages the 4x-slow fp32 PE mode
            w2sr = persist.tile([P, 2 * grp], f32r)
            nc.vector.tensor_copy(out=w2sr[:, 0:grp], in_=sm_sb[:, MH - grp : MH])
            nc.vector.tensor_copy(
                out=w2sr[:, grp : 2 * grp], in_=sm_sb[:, 3 * MH + 1 - grp : 3 * MH + 1]
            )

            # acc tiles — f32r so the final PE matvec accepts them; two
            # independent accumulation chains (even/odd m) per chunk halve
            # the serial DVE tail.  g == 0 writes them fresh (no memset:
            # walrus rejects f32r memset).
            acc = [
                [
                    persist.tile([P, ln], f32r, tag=f"acc{c}_{p}", name=f"acc{c}_{p}")
                    for p in range(grp)
                ]
                for c in range(NC)
            ] if NG > 1 else None

            # --- weights + activations ---
            xtg_sb = persist.tile([P, XW * ln], bf16)
            w1_sb = persist.tile([P, KD * H], bf16)
            if use_q:
                xq_sb = persist.tile([P, NC, kq, ln], fp8)
                wq_sb = persist.tile([P, MH, kq, P], fp8)

            def w1_load(g):
                sl = slice(g * KD * GP, (g + 1) * KD * GP)
                nc.sync.dma_start(out=w1_sb[:, sl], in_=w1_d[:, sl])

            # PE warmup: dependency-free matmuls on scratch data run during
            # the fixed ~6.6us program prologue, flipping the HAM clock-gate
            # to 2.4GHz and covering the DMA wait for the first real pieces
            if NWARM:
                warm = persist.tile([P, ln], bf16, tag="warm", name="warm")
                nc.vector.memset(warm[:], 0.0)
                pwarm = psum_h.tile([P, ln], f32, tag="psh", name="psh")
                for i in range(NWARM):
                    nc.tensor.matmul(
                        pwarm[:],
                        warm[:, 0:P],
                        warm[:],
                        start=(i == 0),
                        stop=(i == NWARM - 1),
                        skip_group_check=True,
                    )

            # each dma_start costs ~650ns of serial HWDGE issue time and a
            # ~2.2-4.6us completion receipt, and queued transfers drain
            # FIFO — few big pieces, most-critical first, all on the SP
            # ring.  Phase A (chunk 0) is gated by xtg chunk 0 + w1 group-
            # by-group; the fp8 planes + later chunks' pieces ride behind
            # and post during phase A (needed only from ~28us).
            # the two phase-A front pieces go down in halves: on cores whose
            # DMA path has a slow episode (~1.5x wire+receipt, seen on 1-2
            # cores in ~half the runs) the first-needed half (k-tiles 0..3)
            # posts ~3us earlier, shrinking the warmup->data gap below the
            # HAM re-throttle threshold
            hx = KD // 2 * ln
            nc.sync.dma_start(out=xtg_sb[:, 0:hx], in_=xtg_d[:, 0:hx])
            hw = KD // 2 * GP
            nc.sync.dma_start(out=w1_sb[:, 0:hw], in_=w1_d[:, 0:hw])
            nc.sync.dma_start(out=xtg_sb[:, hx : KD * ln], in_=xtg_d[:, hx : KD * ln])
            nc.sync.dma_start(out=w1_sb[:, hw : KD * GP], in_=w1_d[:, hw : KD * GP])
            gq = NG // 2  # chunk-0 groups >= gq run DoubleRow
            for g in range(1, min(4, NG)):
                w1_load(g)
            if use_q:
                # fp8 tokens + the g4-5 planes slip into measured phase-A
                # slack just ahead of chunk-0's first DoubleRow group (~22us)
                nc.sync.dma_start(out=xq_sb[:], in_=xq_d[:])
                q4 = (MH // 4) * 2
                nc.sync.dma_start(out=wq_sb[:, q4 : q4 + MH // 4], in_=wq_d[:, q4 : q4 + MH // 4])
            if NG > 4:
                w1_load(4)
            if NG > 5:
                w1_load(5)
            if use_q:
                # g6-7 planes have ~3us of margin to chunk-0's g6 DoubleRow;
                # w1-g5 does not (measured 1.25us gap when it queued behind)
                nc.sync.dma_start(out=wq_sb[:, 3 * MH // 4 :], in_=wq_d[:, 3 * MH // 4 :])
            for g in range(6, NG):
                w1_load(g)
            if use_q:
                nc.sync.dma_start(out=wq_sb[:, 0 : MH // 2], in_=wq_d[:, 0 : MH // 2])
            if NC > 1:
                sl = slice(KD * ln, XW * ln)
                nc.sync.dma_start(out=xtg_sb[:, sl], in_=xtg_d[:, sl])

            # --- mm1 + relu + scaled accumulate, chunk-phase major ---
            z_sb = persist.tile([1, NC * ln], f32)
            for c in range(NC):
                s_all = w2s_c1 if use_q else w2s
                s_red = w2sr[:, grp : 2 * grp] if use_q else w2sr[:, 0:grp]
                pz = psum_z.tile([1, ln], f32, tag="psz", name="psz")
                for g in range(NG):
                    last_g = g == NG - 1
                    q_c = use_q and (c >= 1 or g >= gq)
                    pss = [
                        psum_h.tile([P, ln], f32, tag="psh", name="psh")
                        for _ in range(grp)
                    ]
                    if q_c:
                        for kp in range(kq // 2):
                            for mi in range(grp):
                                nc.tensor.matmul(
                                    pss[mi][:],
                                    wq_sb[:, g * grp + mi, 2 * kp : 2 * kp + 2, :],
                                    xq_sb[:, c, 2 * kp : 2 * kp + 2, :],
                                    start=(kp == 0),
                                    stop=False,
                                    perf_mode=mybir.MatmulPerfMode.DoubleRow,
                                    skip_group_check=True,
                                )
                    kd0 = kq if q_c else 0
                    for kd in range(kd0, KD):
                        base = (g * KD + kd) * GP
                        for mi in range(grp):
                            nc.tensor.matmul(
                                pss[mi][:],
                                w1_sb[:, base + mi * P : base + (mi + 1) * P],
                                xtg_sb[:, xtg_block(c, kd) : xtg_block(c, kd) + ln],
                                start=(kd == kd0 and not q_c),
                                stop=(kd == KD - 1),
                                skip_group_check=True,
                            )
                    if last_g and NG > 1:
                        # the m0..m(NG*grp-3) chains closed during this
                        # group's matmuls — reduce them into pz now, so only
                        # this group's relu outputs remain on the tail
                        for p in range(grp):
                            nc.tensor.matmul(
                                pz[:],
                                ones[:],
                                acc[c][p][:],
                                start=(p == 0),
                                stop=False,
                                skip_group_check=True,
                            )
                    for mi in range(grp):
                        m = g * grp + mi
                        ht = htp.tile([P, ln], f32r, tag="ht", name="ht")
                        if last_g and mi > 0:
                            # the tail's relus serialize on ACT; run this one
                            # on the (idle) DVE in parallel: max(pre + b1, 0)
                            nc.vector.tensor_scalar(
                                out=ht[:],
                                in0=pss[mi][:],
                                scalar1=b1t[:, m : m + 1],
                                scalar2=0.0,
                                op0=mybir.AluOpType.add,
                                op1=mybir.AluOpType.max,
                            )
                        else:
                            nc.scalar.activation(
                                ht[:],
                                pss[mi][:],
                                mybir.ActivationFunctionType.Relu,
                                bias=b1t[:, m : m + 1],
                            )
                        if last_g:
                            # bypass the DVE chain: scale+reduce this m-tile
                            # directly on the PE (w2s column as stationary)
                            nc.tensor.matmul(
                                pz[:],
                                s_red[:, mi : mi + 1],
                                ht[:],
                                start=(NG == 1 and mi == 0),
                                stop=(mi == grp - 1),
                                skip_group_check=True,
                            )
                        elif g == 0:
                            nc.vector.tensor_scalar(
                                out=acc[c][mi][:],
                                in0=ht[:],
                                scalar1=s_all[:, m : m + 1],
                                scalar2=None,
                                op0=mybir.AluOpType.mult,
                            )
                        else:
                            nc.vector.scalar_tensor_tensor(
                                out=acc[c][mi][:],
                                in0=ht[:],
                                scalar=s_all[:, m : m + 1],
                                in1=acc[c][mi][:],
                                op0=mybir.AluOpType.mult,
                                op1=mybir.AluOpType.add,
                            )

                # phase-end store for this chunk (for all but the last chunk
                # this hides under the next phase)
                sl = slice(c * ln, (c + 1) * ln)
                nc.scalar.activation(
                    z_sb[:, sl],
                    pz[:],
                    mybir.ActivationFunctionType.Copy,
                    bias=0.0,
                )
                nc.sync.dma_start(out=z_d[:, sl], in_=z_sb[:, sl])

    nc.compile()
    _BUILD_CACHE[key] = nc
    return nc


def kernel(x, wg, w1, b1, w2, b2, k):
    import ml_dtypes
    from concourse.bass_utils import run_bass_kernel_spmd

    bf16 = ml_dtypes.bfloat16
    fp8np = ml_dtypes.float8_e4m3  # TRN fp8_e4-compatible bits for |v| <= 240
    x = np.asarray(x)
    wg = np.asarray(wg)
    w1 = np.asarray(w1)
    b1 = np.asarray(b1)
    w2 = np.asarray(w2)
    b2 = np.asarray(b2)
    k = int(k)

    B, S, D = x.shape
    E = wg.shape[1]
    H = w1.shape[2]
    T = B * S
    KD = D // P
    MH = H // P
    assert E == N_CORES, f"expert-parallel layout assumes E == 8, got {E}"

    xf = np.ascontiguousarray(x.reshape(T, D), dtype=np.float32)

    # --- gate + top-k routing (host; needed to build the dispatch shards) ---
    logits = xf @ wg.astype(np.float32)
    logits -= logits.max(axis=1, keepdims=True)
    np.exp(logits, out=logits)
    scores = logits / logits.sum(axis=1, keepdims=True)
    if k >= E:
        topi = np.broadcast_to(np.arange(E, dtype=np.int64), (T, E))
    else:
        topi = np.argpartition(-scores, k, axis=1)[:, :k]
    rows = np.arange(T)[:, None]
    topv = scores[rows, topi]

    # per-expert token lists
    idx_e = []
    val_e = []
    for e in range(E):
        tmask, kpos = np.nonzero(topi == e)
        idx_e.append(tmask)
        val_e.append(topv[tmask, kpos].astype(np.float32))
    max_cnt = max(len(i) for i in idx_e)

    # chunk geometry: NC chunks of ln <= 512 tokens (PSUM bank limit)
    NC = max(1, -(-max_cnt // 512))
    ln = _round_up(-(-max_cnt // NC), 4)
    C = NC * ln
    grp = _grp_for(NC)
    NG = MH // grp
    assert D % P == 0 and H % P == 0 and MH % grp == 0, (D, H)

    fold_gate = not b1.any()  # g*relu(u) == relu(g*u) only when b1 == 0
    kq = KQ if (KQ and fold_gate) else 0  # fp8 scale folding needs b1 == 0
    KB = KD - kq
    use_q = kq > 0
    s_e = w2.astype(np.float32).sum(axis=2)          # [E, H] row-sums
    b2s_e = b2.astype(np.float32).sum(axis=1)        # [E]
    sq = SX * SW if kq else 1.0

    nc = _build_program(D, H, ln, NC, kq)

    in_maps = []
    for e in range(E):
        n_e = len(idx_e[e])
        xg = np.zeros((D, C), dtype=np.float32)
        if n_e:
            cols = xf[idx_e[e]].T
            if fold_gate:
                cols = cols * val_e[e][None, :]
            xg[:, :n_e] = cols
        xg4 = xg.reshape(KD, P, NC, ln).transpose(1, 2, 0, 3)  # [P, NC, KD, ln]
        w14 = (
            w1[e].astype(np.float32).reshape(KD, P, MH, P).transpose(1, 2, 0, 3)
        )  # [P, MH, KD, P]
        # xtg: raw tokens; chunk 0 all KD k-tiles, chunks >= 1 k-tiles kq..
        # (the sq scale rides on the bf16 WEIGHTS, which are group-local,
        # so mixed DR/non-DR groups stay PSUM-scale-consistent)
        if use_q:
            parts = [xg4[:, 0].reshape(P, KD * ln)]
            for c in range(1, NC):
                parts.append(xg4[:, c, kq:].reshape(P, KB * ln))
            xtg = np.ascontiguousarray(np.concatenate(parts, axis=1)).astype(bf16)
        else:
            xtg = np.ascontiguousarray(xg4.reshape(P, NC * KD * ln)).astype(bf16)
        # w1 -> [P, KD*H], block (g, kd) of grp*P columns, scaled by sq
        w1r = np.ascontiguousarray(
            (w14 * sq)
            .reshape(P, NG, grp, KD, P)
            .transpose(0, 1, 3, 2, 4)
            .reshape(P, KD * H)
        ).astype(bf16)
        sm = np.zeros((P, 3 * MH + 1 + grp), dtype=np.float32)
        sm[:, 0:MH] = s_e[e].reshape(MH, P).T / sq
        sm[:, MH : 2 * MH] = b1[e].astype(np.float32).reshape(MH, P).T * sq
        sm[:, 2 * MH] = 1.0
        sm[:, 2 * MH + 1 : 3 * MH + 1] = sm[:, 0:MH]
        m = {"xtg": xtg, "w1": w1r, "sm": sm}
        if use_q:
            xqv = np.clip(xg4[:, :, :kq] * SX, -240.0, 240.0)   # [P, NC, kq, ln]
            wqv = np.clip(w14[:, :, :kq] * SW, -240.0, 240.0)   # [P, MH, kq, P]
            m["xq"] = np.ascontiguousarray(xqv).astype(fp8np)
            m["wq"] = np.ascontiguousarray(wqv).astype(fp8np)
        in_maps.append(m)

    res = run_bass_kernel_spmd(nc, in_maps, core_ids=list(range(N_CORES)))

    # --- combine: scatter-add per-(token, expert) scalars, then log_softmax ---
    s = np.zeros(T, dtype=np.float32)
    for e in range(E):
        n_e = len(idx_e[e])
        if n_e:
            z = res.results[e]["z"][0, :n_e].astype(np.float32)
            if fold_gate:
                s[idx_e[e]] += z
            else:
                s[idx_e[e]] += val_e[e] * z
    if b2s_e.any():
        for e in range(E):
            if len(idx_e[e]):
                s[idx_e[e]] += val_e[e] * b2s_e[e]

    sm = s.reshape(B, S)
    sm = sm - sm.max(axis=1, keepdims=True)
    out = sm - np.log(np.exp(sm).sum(axis=1, keepdims=True))
    return out.astype(np.float32)
